# revision 2
# baseline (speedup 1.0000x reference)
"""Performer attention (FAVOR+) TRN2 Bass kernel — v4 (bf16, cycle-scheduled).

Sharding: 8 cores = batch(4) x head-group(2). Core c handles batch c//2,
heads [4*(c%2), 4*(c%2)+4). Each core computes a partial^T [512, 2048] =
Wo_slice^T @ o^T for its head group; host sums partials and adds bo.

Math (exact eps handling; ratio m^-1/2 cancels):
  qT = Wq_s^T x^T + bq ; kT likewise; v token-major (no bv)
  Ek = exp(dd_k - diag_k) [tok, m]; emk = eps*max(e^{dd_k}) via
       max(rowmax(Ek) * e^{diag_k})
  ctx = [v|1]^T Ek + emk*[vsum|T] x 1_m + bv x ksum        [65, m]
  Eq = exp(dd_q) [m, tok]; mq = exact rowmax(dd_q) (token-major pass)
  tq = eps*e^{diag_q+mq};  nd = ctxT' Eq + c0 x tq
  o^T = nd[0:64]/nd[64] ; partial^T = Wo_s^T o^T
All matmul operands bf16 (fp32 PSUM accumulate); rel err ~6e-3 << 2e-2.

Schedule: prologue [proj q/k; sq-k->dkc; sq-q; ph3(0)], then cycle h:
[ek-dd(h) x v/ctx(h-1)] | close(h-1) | [eq-dd(h) x (transp(h-1),
ph3(h+1), nd(h-1))] | stats(h) | tq halves.  Scalar (exp) paces
~17-19us/cycle; Tensor and DVE balance just under it.
"""
import numpy as np


class _Done(Exception):
    pass


T, E, C, D, M = 2048, 512, 256, 64, 512
EPS = 1e-4
LNEPS = float(np.log(EPS))
NCORES = 8

_CACHE = {}


def _build(phase=9, dbg=False):
    import concourse.mybir as mybir
    import concourse.tile as tile
    from concourse import bacc
    from concourse.bass_isa import ReduceOp

    F32 = mybir.dt.float32
    BF16 = mybir.dt.bfloat16
    AF = mybir.ActivationFunctionType
    ALU = mybir.AluOpType
    AX = mybir.AxisListType

    nc = bacc.Bacc("TRN2", target_bir_lowering=False, debug=False,
                   num_devices=NCORES)

    def din(name, shape, dt=BF16):
        return nc.dram_tensor(name, shape, dt, kind="ExternalInput").ap()

    xT_d = din("xT", [128, 4, T])       # host pre-rearranged [p, k, t]
    wq_d = din("wq", [128, 4, C])
    wk_d = din("wk", [128, 4, C])
    wv_d = din("wv", [128, 4, C])
    wo_d = din("wo", [128, 2, E])
    pj_d = din("projT2", [128, M])
    sel_d = din("sel", [128, 4])
    bvc_d = din("bvcol", [1, 260])
    bq_d = din("bq", [C, 1], F32)
    bk_d = din("bk", [C, 1], F32)
    id_d = din("ident", [128, 128], F32)
    pT_d = nc.dram_tensor("pT", [E, T], BF16, kind="ExternalOutput").ap()
    dbg_d = {}
    if dbg:
        for nm, shp, dt_ in [("d_qt", [128, 2, T], BF16), ("d_kt", [128, 2, T], BF16),
                        ("d_vext", [128, 16, 260], BF16), ("d_tq", [4, T], BF16),
                        ("d_rq", [4, T], F32), ("d_mr", [4, T], F32),
                        ("d_dkc", [128, 16, 4], F32), ("d_vsre", [1, 260], F32),
                        ("d_ek0", [128, 16, M], BF16), ("d_eq0", [128, 4, T], BF16),
                        ("d_cs0", [65, 512], F32), ("d_cT0", [128, 4, 66], BF16),
                        ("d_c0s", [2, 65], BF16),
                        ("d_ott", [128, 2, T], BF16), ("d_nd0", [128, 512], F32),
                        ("d_rc0", [1, 512], F32), ("d_db0", [64, 512], F32)]:
            dbg_d[nm] = nc.dram_tensor(nm, shp, dt_, kind="ExternalOutput").ap()

    import contextlib
    with tile.TileContext(nc) as tc:
      with contextlib.suppress(_Done):
        with (
            tc.tile_pool(name="const", bufs=1) as cp,
            tc.tile_pool(name="pers", bufs=1) as pp_,
            tc.tile_pool(name="head", bufs=2) as hp,
            tc.tile_pool(name="smallB", bufs=2) as spB,
            tc.tile_pool(name="dram", bufs=2, space="DRAM") as dp,
            tc.tile_pool(name="pdd", bufs=2, space="PSUM") as pdd,
            tc.tile_pool(name="psm", bufs=2, space="PSUM") as psm,
        ):
            # ---- constants (x first: prologue matmuls wait on it) ----
            xsl = cp.tile([128, 4, T], BF16)
            nc.sync.dma_start(xsl[:, :, 0:512], xT_d[:, :, 0:512])
            wqt = cp.tile([128, 4, C], BF16)
            wkt = cp.tile([128, 4, C], BF16)
            nc.sync.dma_start(wqt[:], wq_d[:])
            nc.sync.dma_start(wkt[:], wk_d[:])
            for nt in range(1, 4):
                nc.sync.dma_start(xsl[:, :, 512 * nt:512 * nt + 512],
                                  xT_d[:, :, 512 * nt:512 * nt + 512])
            wvt = cp.tile([128, 4, C], BF16)
            nc.sync.dma_start(wvt[:], wv_d[:])
            wot = cp.tile([128, 2, E], BF16)
            nc.sync.dma_start(wot[:], wo_d[:])
            pjt = cp.tile([128, M], BF16)
            nc.sync.dma_start(pjt[:], pj_d[:])
            selt = cp.tile([128, 4], BF16)
            nc.sync.dma_start(selt[:], sel_d[:])
            bvc = cp.tile([1, 260], BF16)
            nc.sync.dma_start(bvc[:], bvc_d[:])
            bqt = cp.tile([128, 2, 1], F32)
            nc.sync.dma_start(bqt[:], bq_d.rearrange("(k p) c -> p k c", p=128))
            bkt = cp.tile([128, 2, 1], F32)
            nc.sync.dma_start(bkt[:], bk_d.rearrange("(k p) c -> p k c", p=128))
            idt = cp.tile([128, 128], F32)
            nc.sync.dma_start(idt[:], id_d[:])
            o128 = cp.tile([128, 1], BF16)
            nc.vector.memset(o128[:], 1.0)
            orow = cp.tile([1, M], BF16)
            nc.vector.memset(orow[:], 1.0)
            lne = cp.tile([2, 1], F32)
            nc.vector.memset(lne[:], LNEPS)

            # ---- persistent ----
            qt = pp_.tile([128, 2, T], BF16)
            kt = pp_.tile([128, 2, T], BF16)
            ott = pp_.tile([128, 2, T], BF16)
            vext = pp_.tile([128, 16, 260], BF16)  # [tok128, tt, 65h+(v|1)]
            # q diag/max stats in head-pair halves (partition base 0)
            rqh = [pp_.tile([2, T], F32, name=f"rq{i}") for i in range(2)]
            mrh = [pp_.tile([2, T], F32, name=f"mr{i}") for i in range(2)]
            tqh = [pp_.tile([2, T], BF16, name=f"tq{i}") for i in range(2)]
            dkc = pp_.tile([128, 16, 4], F32)   # -diag_k cols [tok, tt, h]
            vsre = pp_.tile([1, 260], F32)  # eps*[vsum|T] per head slice
            mqT = pp_.tile([16, 128], F32)

            for hh in range(4):
                nc.vector.memset(vext[:, :, 65 * hh + 64:65 * hh + 65], 1.0)

            sqp = tc.alloc_tile_pool(name="sqp", bufs=2)

            # ---------------- emitters ----------------
            def sq_q(pt):
                # q squares -> +diag_q rows (head pair pt)
                for nt in range(4):
                    sl = slice(512 * nt, 512 * nt + 512)
                    sq = sqp.tile([128, 512], BF16, tag="sq")
                    nc.gpsimd.tensor_mul(sq[:], qt[:, pt, sl], qt[:, pt, sl])
                    pd = psm.tile([128, 512], F32, tag="ps")
                    nc.tensor.matmul(pd[0:2, :], selt[:, 0:2], sq[:],
                                     start=True, stop=True)
                    nc.vector.tensor_copy(rqh[pt][:, sl], pd[0:2, :])

            def sq_k(pt):
                # k squares -> -diag_k COLUMNS directly (tiny 2-row matmuls)
                for tg in range(4):
                    sq = sqp.tile([128, 512], BF16, tag="sq")
                    sl = slice(512 * tg, 512 * tg + 512)
                    nc.gpsimd.tensor_mul(sq[:], kt[:, pt, sl], kt[:, pt, sl])
                    pd = psm.tile([128, 512], F32, tag="ps")
                    for j2 in range(4):
                        nc.tensor.matmul(
                            pd[:, 2 * j2:2 * j2 + 2],
                            sq[:, 128 * j2:128 * j2 + 128], selt[:, 2:4],
                            start=True, stop=True)
                    nc.vector.tensor_copy(
                        dkc[:, 4 * tg:4 * tg + 4, 2 * pt:2 * pt + 2],
                        pd[:, 0:8].rearrange("p (a b) -> p a b", b=2))

            def v_block(tt):
                pv = psm.tile([128, 512], F32, tag="ps")
                for k in range(4):
                    nc.tensor.matmul(
                        pv[:, 0:256], xsl[:, k, 128 * tt:128 * tt + 128],
                        wvt[:, k, :], start=(k == 0), stop=(k == 3))
                nc.vector.tensor_copy(
                    vext[:, tt].rearrange("p (g c) -> p g c", c=65)[:, :, 0:64],
                    pv[:, 0:256].rearrange("p (g c) -> p g c", c=64))

            def ph3_start(h):
                return hp.tile([128, 16], F32, tag="mqc", name="mqc")

            def ph3_block(h, mqc, g):
                po, pt = 64 * (h % 2), h // 2
                pq3 = pdd.tile([128, 1024], F32, tag="dd")
                for j in range(2):
                    tt = 2 * g + j
                    nc.tensor.matmul(
                        pq3[:, 512 * j:512 * j + 512],
                        qt[po:po + 64, pt, 128 * tt:128 * tt + 128],
                        pjt[po:po + 64, :], start=True, stop=True)
                nc.vector.tensor_reduce(
                    mqc[:, 2 * g:2 * g + 2],
                    pq3[:].rearrange("p (a b) -> p a b", b=512),
                    axis=AX.X, op=ALU.max)

            def ph3_finish(h, mqc):
                ptm = psm.tile([128, 512], F32, tag="ps")
                nc.tensor.transpose(ptm[0:16, 0:128], mqc[:], idt[:, 0:128])
                nc.vector.tensor_copy(mqT[:], ptm[0:16, 0:128])
                d2 = dp.tile([1, T], F32, tag="d2")
                nc.sync.dma_start(
                    d2.rearrange("a (p f) -> (a p) f", p=16), mqT[:])
                nc.sync.dma_start(mrh[h // 2][h % 2:h % 2 + 1, :], d2[:])

            def tq_half(i):
                nc.gpsimd.tensor_add(mrh[i][:], mrh[i][:], rqh[i][:])
                nc.scalar.activation(tqh[i][:], mrh[i][:], AF.Exp, bias=lne[:])

            def ek_block(h, st, g):
                po, pt = 64 * (h % 2), h // 2
                pk3 = pdd.tile([128, 1024], F32, tag="dd")
                for j in range(2):
                    tt = 2 * g + j
                    nc.tensor.matmul(
                        pk3[:, 512 * j:512 * j + 512],
                        kt[po:po + 64, pt, 128 * tt:128 * tt + 128],
                        pjt[po:po + 64, :], start=True, stop=True)
                for j in range(2):
                    tt = 2 * g + j
                    nc.scalar.activation(
                        st["ek"][:, tt, :], pk3[:, 512 * j:512 * j + 512],
                        AF.Exp, bias=dkc[:, tt, h:h + 1])

            def ctx_block(h, st, g):
                for j in range(2):
                    tt = 2 * g + j
                    nc.tensor.matmul(st["pc"][0:65, :],
                                     vext[:, tt, 65 * h:65 * h + 65],
                                     st["ek"][:, tt, :],
                                     start=(tt == 0), stop=False)

            def stats_chain(h, st):
                ekm = hp.tile([128, 16], BF16, tag="ekm")
                nc.vector.tensor_reduce(ekm[:], st["ek"][:], axis=AX.X,
                                        op=ALU.max)
                ediag = spB.tile([128, 16], F32, tag="ed")
                nc.scalar.activation(ediag[:], dkc[:, :, h], AF.Exp,
                                     scale=-1.0)
                emx2 = spB.tile([128, 16], F32, tag="ex2")
                nc.vector.tensor_mul(emx2[:], ekm[:], ediag[:])
                emx1 = spB.tile([128, 1], F32, tag="ex1")
                nc.vector.tensor_reduce(emx1[:], emx2[:], axis=AX.X,
                                        op=ALU.max)
                emx = hp.tile([128, 1], F32, tag="emx")
                nc.gpsimd.partition_all_reduce(emx[:], emx1[:], channels=128,
                                               reduce_op=ReduceOp.max)
                emv = spB.tile([1, 65], BF16, tag="emv")
                nc.vector.tensor_scalar(emv[:], vsre[0:1, 65 * h:65 * h + 65],
                                        emx[0:1, :], None, ALU.mult)
                st["emv"], st["emx"] = emv, emx

            def ctx_close(h, st):
                pc = st["pc"]
                ksr = spB.tile([1, 512], BF16, tag="ksr")
                kse = spB.tile([1, 1], F32, tag="kse")
                nc.vector.tensor_scalar(kse[:], st["emx"][0:1, :],
                                        float(T * EPS), None, ALU.mult)
                nc.vector.tensor_scalar(ksr[:], pc[64:65, :], kse[0:1, :],
                                        None, ALU.add)
                nc.tensor.matmul(pc[0:65, :], st["emv"][:], orow[:],
                                 start=False, stop=False, skip_group_check=True)
                nc.tensor.matmul(pc[0:65, :], bvc[0:1, 65 * h:65 * h + 65],
                                 ksr[:], start=False, stop=True,
                                 skip_group_check=True)

            def ctx_transpose(h, st):
                cs = hp.tile([65, 512], F32, tag="cs")
                nc.vector.tensor_copy(cs[:], st["pc"][0:65, :])
                cT = hp.tile([128, 4, 66], BF16, tag="cT")
                for mt in range(4):
                    pt2 = psm.tile([128, 512], F32, tag="ps")
                    nc.tensor.transpose(pt2[:, 0:65],
                                        cs[:, 128 * mt:128 * mt + 128],
                                        idt[0:65, 0:65])
                    nc.vector.tensor_copy(cT[:, mt, 0:65], pt2[:, 0:65])
                pc0 = psm.tile([128, 512], F32, tag="ps")
                for mt in range(4):
                    nc.tensor.matmul(pc0[0:1, 0:65], o128[:], cT[:, mt, 0:65],
                                     start=(mt == 0), stop=(mt == 3))
                c0s = hp.tile([2, 65], BF16, tag="c0s")
                nc.vector.memset(c0s[:], 0.0)
                if h % 2 == 0:
                    nc.vector.tensor_copy(c0s[0:1, :], pc0[0:1, 0:65])
                else:
                    c0b = spB.tile([1, 65], BF16, tag="c0b")
                    nc.vector.tensor_copy(c0b[:], pc0[0:1, 0:65])
                    nc.sync.dma_start(c0s[1:2, :], c0b[:])
                st["cs"], st["cT"], st["c0s"] = cs, cT, c0s

            def eq_block(h, st, i):
                # one 1024-wide matmul (shared stationary for both halves)
                po, pt = 64 * (h % 2), h // 2
                mt, gg = i // 2, i % 2
                pq1 = pdd.tile([128, 1024], F32, tag="dd")
                for j in range(2):
                    ntt = 2 * gg + j
                    nc.tensor.matmul(
                        pq1[:, 512 * j:512 * j + 512],
                        pjt[po:po + 64, 128 * mt:128 * mt + 128],
                        qt[po:po + 64, pt, 512 * ntt:512 * ntt + 512],
                        start=True, stop=True)
                nc.scalar.activation(
                    st["eq"][:, mt, 1024 * gg:1024 * gg + 1024], pq1[:],
                    AF.Exp)

            def nd_block(h, st, nt):
                po, pt = 64 * (h % 2), h // 2
                pn = psm.tile([128, 512], F32, tag="ps")
                for mt in range(4):
                    nc.tensor.matmul(pn[0:65, :], st["cT"][:, mt, 0:65],
                                     st["eq"][:, mt, 512 * nt:512 * nt + 512],
                                     start=(mt == 0), stop=False)
                nc.tensor.matmul(pn[0:65, :], st["c0s"][:],
                                 tqh[h // 2][:, 512 * nt:512 * nt + 512],
                                 start=False, stop=True)
                den = spB.tile([1, 512], F32, tag="den")
                nc.vector.tensor_copy(den[:], pn[64:65, :])
                rc = spB.tile([1, 512], F32, tag="rc")
                nc.vector.reciprocal_approx_fast(rc[:], den[:])
                db = spB.tile([64, 512], F32, tag="db")
                nc.gpsimd.partition_broadcast(db[:], rc[:], channels=64)
                if dbg and h == 0 and nt == 0:
                    ndv = spB.tile([128, 512], F32, tag="ndv")
                    nc.vector.tensor_copy(ndv[:], pn[:])
                    nc.sync.dma_start(dbg_d["d_nd0"], ndv[:])
                    nc.sync.dma_start(dbg_d["d_rc0"], rc[:])
                    nc.sync.dma_start(dbg_d["d_db0"], db[:])
                    nc.sync.dma_start(dbg_d["d_c0s"], st["c0s"][:])
                nc.vector.tensor_mul(
                    ott[po:po + 64, pt, 512 * nt:512 * nt + 512],
                    pn[0:64, :], db[:])

            def op_block(nt2):
                # nt2 is a pair of nt chunks; [128,1024] psum per et
                for et in range(4):
                    pw = pdd.tile([128, 1024], F32, tag="dd")
                    for j in range(2):
                        nt = 2 * nt2 + j
                        for k2 in range(2):
                            nc.tensor.matmul(
                                pw[:, 512 * j:512 * j + 512],
                                wot[:, k2, 128 * et:128 * et + 128],
                                ott[:, k2, 512 * nt:512 * nt + 512],
                                start=(k2 == 0), stop=(k2 == 1))
                    wev = spB.tile([128, 1024], BF16, tag="wev")
                    if et % 2 == 0:
                        nc.scalar.copy(wev[:], pw[:])
                    else:
                        nc.vector.tensor_copy(wev[:], pw[:])
                    nc.sync.dma_start(
                        pT_d[128 * et:128 * et + 128,
                             1024 * nt2:1024 * nt2 + 1024],
                        wev[:])

            def new_st():
                return {
                    "ek": hp.tile([128, 16, M], BF16, tag="ek", name="ek"),
                    "eq": hp.tile([128, 4, T], BF16, tag="eq", name="eq"),
                    "pc": psm.tile([128, 512], F32, tag="ctx", bufs=2,
                                   name="pc"),
                }

            # ---------------- prologue ----------------
            for nt in range(4):
                pq_ = pdd.tile([128, 1024], F32, tag="dd")
                pk_ = pdd.tile([128, 1024], F32, tag="dd")
                for k in range(4):
                    xc = xsl[:, k, 512 * nt:512 * nt + 512]
                    for ct_ in range(2):
                        nc.tensor.matmul(
                            pq_[:, 512 * ct_:512 * ct_ + 512],
                            wqt[:, k, 128 * ct_:128 * ct_ + 128], xc,
                            start=(k == 0), stop=(k == 3))
                        nc.tensor.matmul(
                            pk_[:, 512 * ct_:512 * ct_ + 512],
                            wkt[:, k, 128 * ct_:128 * ct_ + 128], xc,
                            start=(k == 0), stop=(k == 3))
                for ct_ in range(2):
                    nc.scalar.activation(
                        qt[:, ct_, 512 * nt:512 * nt + 512],
                        pq_[:, 512 * ct_:512 * ct_ + 512],
                        AF.Identity, bias=bqt[:, ct_, :])
                    nc.scalar.activation(
                        kt[:, ct_, 512 * nt:512 * nt + 512],
                        pk_[:, 512 * ct_:512 * ct_ + 512],
                        AF.Identity, bias=bkt[:, ct_, :])
            sq_k(0)
            sq_k(1)
            mqc_n = ph3_start(0)
            for g in range(8):
                ph3_block(0, mqc_n, g)
                if g == 3:
                    sq_q(0)
                elif g == 7:
                    sq_q(1)
            ph3_finish(0, mqc_n)
            if phase < 2:
                raise _Done

            # ---------------- head cycles ----------------
            # cycle h partA: ek-dd(h) x [v (h=0) | close(h-1), transp(h-1),
            #                nd(h-1)];  partB: eq-dd(h) x ctx(h) x ph3(h+1)
            nheads = 4 if phase >= 5 else 1
            prev = None
            for h in range(nheads):
                st = new_st()
                for g in range(8):
                    ek_block(h, st, g)
                    if h == 0:
                        v_block(2 * g)
                        v_block(2 * g + 1)
                    else:
                        if g == 1:
                            ctx_close(h - 1, prev)
                        elif g == 2:
                            ctx_transpose(h - 1, prev)
                        elif g >= 4:
                            nd_block(h - 1, prev, g - 4)
                if h == 0:
                    # vsum row (eps-scaled) — needs all v blocks
                    ps = psm.tile([128, 512], F32, tag="ps")
                    for tt in range(16):
                        nc.tensor.matmul(ps[0:1, 0:260], o128[:],
                                         vext[:, tt, :],
                                         start=(tt == 0), stop=(tt == 15))
                    nc.vector.tensor_scalar(vsre[:], ps[0:1, 0:260], EPS,
                                            None, ALU.mult)
                if h < nheads - 1:
                    mqc_n = ph3_start(h + 1)
                for i in range(8):
                    eq_block(h, st, i)
                    ctx_block(h, st, i)
                    if h < nheads - 1:
                        ph3_block(h + 1, mqc_n, i)
                if h < nheads - 1:
                    ph3_finish(h + 1, mqc_n)
                stats_chain(h, st)
                if h == 0:
                    tq_half(0)   # mr rows 0 (prologue) + 1 (this cycle)
                elif h == 2:
                    tq_half(1)   # mr rows 2 + 3
                prev = st
            # tail: close/transpose(last) + nd(last) x outproj
            h = nheads - 1
            ctx_close(h, prev)
            ctx_transpose(h, prev)
            if dbg:
                nc.sync.dma_start(dbg_d["d_cs0"], prev["cs"][:])
                nc.sync.dma_start(dbg_d["d_cT0"], prev["cT"][:])
                nc.sync.dma_start(dbg_d["d_ek0"], prev["ek"][:])
                nc.sync.dma_start(dbg_d["d_eq0"], prev["eq"][:])
            if phase < 6:
                for nt in range(4):
                    nd_block(h, prev, nt)
            else:
                nd_block(h, prev, 0)
                nd_block(h, prev, 1)
                op_block(0)
                nd_block(h, prev, 2)
                nd_block(h, prev, 3)
                op_block(1)

            if dbg:
                dr = dp.tile([4, T], F32, tag="dr")
                for i in range(2):
                    nc.sync.dma_start(dr[2 * i:2 * i + 2, :], mrh[i][:])
                nc.sync.dma_start(dbg_d["d_mr"], dr[:])
                dr2 = dp.tile([4, T], F32, tag="dr2")
                for i in range(2):
                    nc.sync.dma_start(dr2[2 * i:2 * i + 2, :], rqh[i][:])
                nc.sync.dma_start(dbg_d["d_rq"], dr2[:])
                dr3 = dp.tile([4, T], BF16, tag="dr3")
                for i in range(2):
                    nc.sync.dma_start(dr3[2 * i:2 * i + 2, :], tqh[i][:])
                nc.sync.dma_start(dbg_d["d_tq"], dr3[:])
                for nm, tile_ in (("d_qt", qt), ("d_kt", kt), ("d_vext", vext),
                                  ("d_dkc", dkc), ("d_vsre", vsre),
                                  ("d_ott", ott)):
                    nc.sync.dma_start(dbg_d[nm], tile_[:])
            sqp.release()
    nc.compile()
    return nc


def _prep_inputs(x, Wq, bq, Wk, bk, Wv, bv, Wo, bo, proj):
    import ml_dtypes
    bf16 = ml_dtypes.bfloat16
    dn = float(D) ** -0.25
    projT_dn = np.ascontiguousarray((dn * proj).T).astype(np.float32)  # [D, M]
    projT2 = np.concatenate([projT_dn, projT_dn], 0).astype(bf16)      # [128, M]
    sel = np.zeros((128, 4), np.float32)
    sel[0:64, 0] = 0.0625
    sel[64:128, 1] = 0.0625
    sel[0:64, 2] = -0.0625
    sel[64:128, 3] = -0.0625
    ident = np.eye(128, dtype=np.float32)
    common = {
        "projT2": projT2,
        "sel": sel.astype(bf16),
        "ident": ident,
    }
    in_maps = []
    for c in range(NCORES):
        b, hg = c // 2, c % 2
        sl = slice(C * hg, C * hg + C)
        bvs = bv[sl]
        bvcol = np.zeros((1, 260), np.float32)
        for h in range(4):
            bvcol[0, 65 * h:65 * h + 64] = bvs[64 * h:64 * h + 64]
        m = dict(common)
        m["xT"] = np.ascontiguousarray(
            x[b].T.reshape(4, 128, T).transpose(1, 0, 2)).astype(bf16)
        m["wq"] = np.ascontiguousarray(
            Wq[:, sl].reshape(4, 128, C).transpose(1, 0, 2)).astype(bf16)
        m["wk"] = np.ascontiguousarray(
            Wk[:, sl].reshape(4, 128, C).transpose(1, 0, 2)).astype(bf16)
        m["wv"] = np.ascontiguousarray(
            Wv[:, sl].reshape(4, 128, C).transpose(1, 0, 2)).astype(bf16)
        m["wo"] = np.ascontiguousarray(
            Wo[sl, :].reshape(2, 128, E).transpose(1, 0, 2)).astype(bf16)
        m["bq"] = np.ascontiguousarray(bq[sl].reshape(C, 1)).astype(np.float32)
        m["bk"] = np.ascontiguousarray(bk[sl].reshape(C, 1)).astype(np.float32)
        m["bvcol"] = bvcol.astype(bf16)
        in_maps.append(m)
    return in_maps


def kernel(x, Wq, bq, Wk, bk, Wv, bv, Wo, bo, proj, _trace=False):
    from concourse.bass_utils import run_bass_kernel_spmd

    x = np.asarray(x, np.float32)
    args = [np.asarray(a, np.float32) for a in (Wq, bq, Wk, bk, Wv, bv, Wo, bo, proj)]
    Wq, bq, Wk, bk, Wv, bv, Wo, bo, proj = args

    if "nc" not in _CACHE:
        _CACHE["nc"] = _build()
    nc = _CACHE["nc"]

    in_maps = _prep_inputs(x, Wq, bq, Wk, bk, Wv, bv, Wo, bo, proj)
    res = run_bass_kernel_spmd(nc, in_maps, list(range(NCORES)), trace=_trace)
    out = np.zeros((4, T, E), np.float32)
    for c in range(NCORES):
        out[c // 2] += np.asarray(res.results[c]["pT"], np.float32).T
    out += bo[None, None, :]
    if _trace:
        return out, res
    return out


# revision 3
# speedup vs baseline: 1.0306x; 1.0306x over previous
"""Performer attention (FAVOR+) TRN2 Bass kernel — v4 (bf16, cycle-scheduled).

Sharding: 8 cores = batch(4) x head-group(2). Core c handles batch c//2,
heads [4*(c%2), 4*(c%2)+4). Each core computes a partial^T [512, 2048] =
Wo_slice^T @ o^T for its head group; host sums partials and adds bo.

Math (exact eps handling; ratio m^-1/2 cancels):
  qT = Wq_s^T x^T + bq ; kT likewise; v token-major (no bv)
  Ek = exp(dd_k - diag_k) [tok, m]; emk = eps*max(e^{dd_k}) via
       max(rowmax(Ek) * e^{diag_k})
  ctx = [v|1]^T Ek + emk*[vsum|T] x 1_m + bv x ksum        [65, m]
  Eq = exp(dd_q) [m, tok]; mq = exact rowmax(dd_q) (token-major pass)
  tq = eps*e^{diag_q+mq};  nd = ctxT' Eq + c0 x tq
  o^T = nd[0:64]/nd[64] ; partial^T = Wo_s^T o^T
All matmul operands bf16 (fp32 PSUM accumulate); rel err ~6e-3 << 2e-2.

Schedule: prologue [proj q/k; sq-k->dkc; sq-q; ph3(0)], then cycle h:
[ek-dd(h) x v/ctx(h-1)] | close(h-1) | [eq-dd(h) x (transp(h-1),
ph3(h+1), nd(h-1))] | stats(h) | tq halves.  Scalar (exp) paces
~17-19us/cycle; Tensor and DVE balance just under it.
"""
import numpy as np


class _Done(Exception):
    pass


T, E, C, D, M = 2048, 512, 256, 64, 512
EPS = 1e-4
LNEPS = float(np.log(EPS))
NCORES = 8

_CACHE = {}


def _build(phase=9, dbg=False):
    import concourse.mybir as mybir
    import concourse.tile as tile
    from concourse import bacc
    from concourse.bass_isa import ReduceOp

    F32 = mybir.dt.float32
    BF16 = mybir.dt.bfloat16
    AF = mybir.ActivationFunctionType
    ALU = mybir.AluOpType
    AX = mybir.AxisListType

    nc = bacc.Bacc("TRN2", target_bir_lowering=False, debug=False,
                   num_devices=NCORES)

    def din(name, shape, dt=BF16):
        return nc.dram_tensor(name, shape, dt, kind="ExternalInput").ap()

    xT_d = din("xT", [128, 4, T])       # host pre-rearranged [p, k, t]
    wq_d = din("wq", [128, 4, C])
    wk_d = din("wk", [128, 4, C])
    wv_d = din("wv", [128, 4, C])
    wo_d = din("wo", [128, 2, E])
    pj_d = din("projT2", [128, M])
    sel_d = din("sel", [128, 4])
    bvc_d = din("bvcol", [1, 260])
    bq_d = din("bq", [C, 1], F32)
    bk_d = din("bk", [C, 1], F32)
    id_d = din("ident", [128, 128], F32)
    pT_d = nc.dram_tensor("pT", [E, T], BF16, kind="ExternalOutput").ap()
    dbg_d = {}
    if dbg:
        for nm, shp, dt_ in [("d_qt", [128, 2, T], BF16), ("d_kt", [128, 2, T], BF16),
                        ("d_vext", [128, 16, 260], BF16), ("d_tq", [4, T], BF16),
                        ("d_rq", [4, T], F32), ("d_mr", [4, T], F32),
                        ("d_dkc", [128, 16, 4], F32), ("d_vsre", [1, 260], F32),
                        ("d_ek0", [128, 16, M], BF16), ("d_eq0", [128, 4, T], BF16),
                        ("d_cs0", [65, 512], F32), ("d_cT0", [128, 4, 66], BF16),
                        ("d_c0s", [2, 65], BF16),
                        ("d_ott", [128, 2, T], BF16), ("d_nd0", [128, 512], F32),
                        ("d_rc0", [1, 512], F32), ("d_db0", [64, 512], F32)]:
            dbg_d[nm] = nc.dram_tensor(nm, shp, dt_, kind="ExternalOutput").ap()

    import contextlib
    with tile.TileContext(nc) as tc:
      with contextlib.suppress(_Done):
        with (
            tc.tile_pool(name="const", bufs=1) as cp,
            tc.tile_pool(name="pers", bufs=1) as pp_,
            tc.tile_pool(name="head", bufs=2) as hp,
            tc.tile_pool(name="smallB", bufs=2) as spB,
            tc.tile_pool(name="dram", bufs=2, space="DRAM") as dp,
            tc.tile_pool(name="pdd", bufs=2, space="PSUM") as pdd,
            tc.tile_pool(name="psm", bufs=2, space="PSUM") as psm,
        ):
            # ---- constants (x first: prologue matmuls wait on it) ----
            xsl = cp.tile([128, 4, T], BF16)
            nc.sync.dma_start(xsl[:, :, 0:512], xT_d[:, :, 0:512])
            wqt = cp.tile([128, 4, C], BF16)
            wkt = cp.tile([128, 4, C], BF16)
            nc.sync.dma_start(wqt[:], wq_d[:])
            nc.sync.dma_start(wkt[:], wk_d[:])
            bqt = cp.tile([128, 2, 1], F32)
            nc.sync.dma_start(bqt[:], bq_d.rearrange("(k p) c -> p k c", p=128))
            bkt = cp.tile([128, 2, 1], F32)
            nc.sync.dma_start(bkt[:], bk_d.rearrange("(k p) c -> p k c", p=128))
            for nt in range(1, 4):
                nc.sync.dma_start(xsl[:, :, 512 * nt:512 * nt + 512],
                                  xT_d[:, :, 512 * nt:512 * nt + 512])
            wvt = cp.tile([128, 4, C], BF16)
            nc.sync.dma_start(wvt[:], wv_d[:])
            wot = cp.tile([128, 2, E], BF16)
            nc.sync.dma_start(wot[:], wo_d[:])
            pjt = cp.tile([128, M], BF16)
            nc.sync.dma_start(pjt[:], pj_d[:])
            selt = cp.tile([128, 4], BF16)
            nc.sync.dma_start(selt[:], sel_d[:])
            bvc = cp.tile([1, 260], BF16)
            nc.sync.dma_start(bvc[:], bvc_d[:])
            idt = cp.tile([128, 128], F32)
            nc.sync.dma_start(idt[:], id_d[:])
            o128 = cp.tile([128, 1], BF16)
            nc.vector.memset(o128[:], 1.0)
            orow = cp.tile([1, M], BF16)
            nc.vector.memset(orow[:], 1.0)
            lne = cp.tile([2, 1], F32)
            nc.vector.memset(lne[:], LNEPS)

            # ---- persistent ----
            qt = pp_.tile([128, 2, T], BF16)
            kt = pp_.tile([128, 2, T], BF16)
            ott = pp_.tile([128, 2, T], BF16)
            vext = pp_.tile([128, 16, 260], BF16)  # [tok128, tt, 65h+(v|1)]
            # q diag/max stats in head-pair halves (partition base 0)
            rqh = [pp_.tile([2, T], F32, name=f"rq{i}") for i in range(2)]
            mrh = [pp_.tile([2, T], F32, name=f"mr{i}") for i in range(2)]
            tqh = [pp_.tile([2, T], BF16, name=f"tq{i}") for i in range(2)]
            dkc = pp_.tile([128, 16, 4], F32)   # -diag_k cols [tok, tt, h]
            vsre = pp_.tile([1, 260], F32)  # eps*[vsum|T] per head slice
            mqT = pp_.tile([16, 128], F32)

            for hh in range(4):
                nc.vector.memset(vext[:, :, 65 * hh + 64:65 * hh + 65], 1.0)

            sqp = tc.alloc_tile_pool(name="sqp", bufs=2)

            # ---------------- emitters ----------------
            def _mul(out, a, b, use_dve):
                if use_dve:
                    nc.vector.tensor_mul(out, a, b)
                else:
                    nc.gpsimd.tensor_mul(out, a, b)

            def sq_q(c):
                # q squares -> +diag_q rows, one 512-col chunk
                pt, nt = c // 4, c % 4
                sl = slice(512 * nt, 512 * nt + 512)
                sq = sqp.tile([128, 512], BF16, tag="sq")
                _mul(sq[:], qt[:, pt, sl], qt[:, pt, sl], c % 2 == 0)
                pd = psm.tile([128, 512], F32, tag="ps")
                nc.tensor.matmul(pd[0:2, :], selt[:, 0:2], sq[:],
                                 start=True, stop=True)
                nc.vector.tensor_copy(rqh[pt][:, sl], pd[0:2, :])

            def sq_k(c):
                # k squares -> -diag_k COLUMNS directly, one chunk
                pt, tg = c // 4, c % 4
                sq = sqp.tile([128, 512], BF16, tag="sq")
                sl = slice(512 * tg, 512 * tg + 512)
                _mul(sq[:], kt[:, pt, sl], kt[:, pt, sl], c % 2 == 1)
                pd = psm.tile([128, 512], F32, tag="ps")
                for j2 in range(4):
                    nc.tensor.matmul(
                        pd[:, 2 * j2:2 * j2 + 2],
                        sq[:, 128 * j2:128 * j2 + 128], selt[:, 2:4],
                        start=True, stop=True)
                nc.vector.tensor_copy(
                    dkc[:, 4 * tg:4 * tg + 4, 2 * pt:2 * pt + 2],
                    pd[:, 0:8].rearrange("p (a b) -> p a b", b=2))

            def v_block(tt):
                pv = psm.tile([128, 512], F32, tag="ps")
                for k in range(4):
                    nc.tensor.matmul(
                        pv[:, 0:256], xsl[:, k, 128 * tt:128 * tt + 128],
                        wvt[:, k, :], start=(k == 0), stop=(k == 3))
                nc.vector.tensor_copy(
                    vext[:, tt].rearrange("p (g c) -> p g c", c=65)[:, :, 0:64],
                    pv[:, 0:256].rearrange("p (g c) -> p g c", c=64))

            def ph3_start(h):
                return hp.tile([128, 16], F32, tag="mqc", name="mqc")

            def ph3_block(h, mqc, g):
                po, pt = 64 * (h % 2), h // 2
                pq3 = pdd.tile([128, 1024], F32, tag="dd")
                for j in range(2):
                    tt = 2 * g + j
                    nc.tensor.matmul(
                        pq3[:, 512 * j:512 * j + 512],
                        qt[po:po + 64, pt, 128 * tt:128 * tt + 128],
                        pjt[po:po + 64, :], start=True, stop=True)
                nc.vector.tensor_reduce(
                    mqc[:, 2 * g:2 * g + 2],
                    pq3[:].rearrange("p (a b) -> p a b", b=512),
                    axis=AX.X, op=ALU.max)

            def ph3_finish(h, mqc):
                ptm = psm.tile([128, 512], F32, tag="ps")
                nc.tensor.transpose(ptm[0:16, 0:128], mqc[:], idt[:, 0:128])
                nc.vector.tensor_copy(mqT[:], ptm[0:16, 0:128])
                d2 = dp.tile([1, T], F32, tag="d2")
                nc.sync.dma_start(
                    d2.rearrange("a (p f) -> (a p) f", p=16), mqT[:])
                nc.sync.dma_start(mrh[h // 2][h % 2:h % 2 + 1, :], d2[:])

            def tq_half(i):
                nc.gpsimd.tensor_add(mrh[i][:], mrh[i][:], rqh[i][:])
                nc.scalar.activation(tqh[i][:], mrh[i][:], AF.Exp, bias=lne[:])

            def ek_block(h, st, g):
                po, pt = 64 * (h % 2), h // 2
                pk3 = pdd.tile([128, 1024], F32, tag="dd")
                for j in range(2):
                    tt = 2 * g + j
                    nc.tensor.matmul(
                        pk3[:, 512 * j:512 * j + 512],
                        kt[po:po + 64, pt, 128 * tt:128 * tt + 128],
                        pjt[po:po + 64, :], start=True, stop=True)
                # raw-dd max -> Mk stats, spread across partA (PSUM holds
                # dd_k pre-diag, so this IS max(dd_k) per partition)
                nc.vector.tensor_reduce(
                    st["kst"][:, 2 * g:2 * g + 2],
                    pk3[:].rearrange("p (a b) -> p a b", b=512),
                    axis=AX.X, op=ALU.max)
                for j in range(2):
                    tt = 2 * g + j
                    nc.scalar.activation(
                        st["ek"][:, tt, :], pk3[:, 512 * j:512 * j + 512],
                        AF.Exp, bias=dkc[:, tt, h:h + 1])

            def ctx_block(h, st, g):
                for j in range(2):
                    tt = 2 * g + j
                    nc.tensor.matmul(st["pc"][0:65, :],
                                     vext[:, tt, 65 * h:65 * h + 65],
                                     st["ek"][:, tt, :],
                                     start=(tt == 0), stop=False)

            def stats_chain(h, st):
                # combine per-g raw-dd maxes -> global Mk -> emk = eps*e^Mk
                emx1 = spB.tile([128, 1], F32, tag="ex1")
                nc.vector.tensor_reduce(emx1[:], st["kst"][:], axis=AX.X,
                                        op=ALU.max)
                emx = hp.tile([128, 1], F32, tag="emx")
                nc.gpsimd.partition_all_reduce(emx[:], emx1[:], channels=128,
                                               reduce_op=ReduceOp.max)
                emk = hp.tile([1, 1], F32, tag="emk")
                nc.scalar.activation(emk[:], emx[0:1, :], AF.Exp,
                                     bias=lne[0:1, :])
                emv = spB.tile([1, 65], BF16, tag="emv")
                nc.vector.tensor_scalar(emv[:], vsre[0:1, 65 * h:65 * h + 65],
                                        emk[0:1, :], None, ALU.mult)
                st["emv"], st["emx"] = emv, emk

            def ctx_close(h, st):
                pc = st["pc"]
                ksr = spB.tile([1, 512], BF16, tag="ksr")
                kse = spB.tile([1, 1], F32, tag="kse")
                nc.vector.tensor_scalar(kse[:], st["emx"][0:1, :],
                                        float(T), None, ALU.mult)
                nc.vector.tensor_scalar(ksr[:], pc[64:65, :], kse[0:1, :],
                                        None, ALU.add)
                nc.tensor.matmul(pc[0:65, :], st["emv"][:], orow[:],
                                 start=False, stop=False, skip_group_check=True)
                nc.tensor.matmul(pc[0:65, :], bvc[0:1, 65 * h:65 * h + 65],
                                 ksr[:], start=False, stop=True,
                                 skip_group_check=True)

            def ctx_transpose(h, st):
                cs = hp.tile([65, 512], F32, tag="cs")
                nc.vector.tensor_copy(cs[:], st["pc"][0:65, :])
                cT = hp.tile([128, 4, 66], BF16, tag="cT")
                for mt in range(4):
                    pt2 = psm.tile([128, 512], F32, tag="ps")
                    nc.tensor.transpose(pt2[:, 0:65],
                                        cs[:, 128 * mt:128 * mt + 128],
                                        idt[0:65, 0:65])
                    nc.vector.tensor_copy(cT[:, mt, 0:65], pt2[:, 0:65])
                pc0 = psm.tile([128, 512], F32, tag="ps")
                for mt in range(4):
                    nc.tensor.matmul(pc0[0:1, 0:65], o128[:], cT[:, mt, 0:65],
                                     start=(mt == 0), stop=(mt == 3))
                c0s = hp.tile([2, 65], BF16, tag="c0s")
                nc.vector.memset(c0s[:], 0.0)
                if h % 2 == 0:
                    nc.vector.tensor_copy(c0s[0:1, :], pc0[0:1, 0:65])
                else:
                    c0b = spB.tile([1, 65], BF16, tag="c0b")
                    nc.vector.tensor_copy(c0b[:], pc0[0:1, 0:65])
                    nc.sync.dma_start(c0s[1:2, :], c0b[:])
                st["cs"], st["cT"], st["c0s"] = cs, cT, c0s

            def eq_block(h, st, i):
                # one 1024-wide matmul (shared stationary for both halves)
                po, pt = 64 * (h % 2), h // 2
                mt, gg = i // 2, i % 2
                pq1 = pdd.tile([128, 1024], F32, tag="dd")
                for j in range(2):
                    ntt = 2 * gg + j
                    nc.tensor.matmul(
                        pq1[:, 512 * j:512 * j + 512],
                        pjt[po:po + 64, 128 * mt:128 * mt + 128],
                        qt[po:po + 64, pt, 512 * ntt:512 * ntt + 512],
                        start=True, stop=True)
                nc.scalar.activation(
                    st["eq"][:, mt, 1024 * gg:1024 * gg + 1024], pq1[:],
                    AF.Exp)

            def nd_block(h, st, nt):
                po, pt = 64 * (h % 2), h // 2
                pn = psm.tile([128, 512], F32, tag="ps")
                for mt in range(4):
                    nc.tensor.matmul(pn[0:65, :], st["cT"][:, mt, 0:65],
                                     st["eq"][:, mt, 512 * nt:512 * nt + 512],
                                     start=(mt == 0), stop=False)
                nc.tensor.matmul(pn[0:65, :], st["c0s"][:],
                                 tqh[h // 2][:, 512 * nt:512 * nt + 512],
                                 start=False, stop=True)
                den = spB.tile([1, 512], F32, tag="den")
                nc.vector.tensor_copy(den[:], pn[64:65, :])
                rc = spB.tile([1, 512], F32, tag="rc")
                nc.vector.reciprocal_approx_fast(rc[:], den[:])
                db = spB.tile([64, 512], F32, tag="db")
                nc.gpsimd.partition_broadcast(db[:], rc[:], channels=64)
                if dbg and h == 0 and nt == 0:
                    ndv = spB.tile([128, 512], F32, tag="ndv")
                    nc.vector.tensor_copy(ndv[:], pn[:])
                    nc.sync.dma_start(dbg_d["d_nd0"], ndv[:])
                    nc.sync.dma_start(dbg_d["d_rc0"], rc[:])
                    nc.sync.dma_start(dbg_d["d_db0"], db[:])
                    nc.sync.dma_start(dbg_d["d_c0s"], st["c0s"][:])
                nc.vector.tensor_mul(
                    ott[po:po + 64, pt, 512 * nt:512 * nt + 512],
                    pn[0:64, :], db[:])

            def op_block(nt2):
                # nt2 is a pair of nt chunks; [128,1024] psum per et
                for et in range(4):
                    pw = pdd.tile([128, 1024], F32, tag="dd")
                    for j in range(2):
                        nt = 2 * nt2 + j
                        for k2 in range(2):
                            nc.tensor.matmul(
                                pw[:, 512 * j:512 * j + 512],
                                wot[:, k2, 128 * et:128 * et + 128],
                                ott[:, k2, 512 * nt:512 * nt + 512],
                                start=(k2 == 0), stop=(k2 == 1))
                    wev = spB.tile([128, 1024], BF16, tag="wev")
                    if et % 2 == 0:
                        nc.scalar.copy(wev[:], pw[:])
                    else:
                        nc.vector.tensor_copy(wev[:], pw[:])
                    nc.sync.dma_start(
                        pT_d[128 * et:128 * et + 128,
                             1024 * nt2:1024 * nt2 + 1024],
                        wev[:])

            def new_st():
                return {
                    "ek": hp.tile([128, 16, M], BF16, tag="ek", name="ek"),
                    "eq": hp.tile([128, 4, T], BF16, tag="eq", name="eq"),
                    "kst": hp.tile([128, 16], F32, tag="kst", name="kst"),
                    "pc": psm.tile([128, 512], F32, tag="ctx", bufs=2,
                                   name="pc"),
                }

            # ---------------- prologue ----------------
            for nt in range(4):
                pq_ = pdd.tile([128, 1024], F32, tag="dd")
                pk_ = pdd.tile([128, 1024], F32, tag="dd")
                for k in range(4):
                    xc = xsl[:, k, 512 * nt:512 * nt + 512]
                    for ct_ in range(2):
                        nc.tensor.matmul(
                            pq_[:, 512 * ct_:512 * ct_ + 512],
                            wqt[:, k, 128 * ct_:128 * ct_ + 128], xc,
                            start=(k == 0), stop=(k == 3))
                        nc.tensor.matmul(
                            pk_[:, 512 * ct_:512 * ct_ + 512],
                            wkt[:, k, 128 * ct_:128 * ct_ + 128], xc,
                            start=(k == 0), stop=(k == 3))
                for ct_ in range(2):
                    nc.scalar.activation(
                        qt[:, ct_, 512 * nt:512 * nt + 512],
                        pq_[:, 512 * ct_:512 * ct_ + 512],
                        AF.Identity, bias=bqt[:, ct_, :])
                    nc.scalar.activation(
                        kt[:, ct_, 512 * nt:512 * nt + 512],
                        pk_[:, 512 * ct_:512 * ct_ + 512],
                        AF.Identity, bias=bkt[:, ct_, :])
            mqc_n = ph3_start(0)
            for g in range(8):
                ph3_block(0, mqc_n, g)
                sq_k(g)
                sq_q(g)
            ph3_finish(0, mqc_n)
            if phase < 2:
                raise _Done

            # ---------------- head cycles ----------------
            # cycle h partA: ek-dd(h) x [v (h=0) | close(h-1), transp(h-1),
            #                nd(h-1)];  partB: eq-dd(h) x ctx(h) x ph3(h+1)
            nheads = 4 if phase >= 5 else 1
            prev = None
            for h in range(nheads):
                st = new_st()
                for g in range(8):
                    ek_block(h, st, g)
                    if h == 0:
                        v_block(2 * g)
                        v_block(2 * g + 1)
                    else:
                        if g == 1:
                            ctx_close(h - 1, prev)
                        elif g == 2:
                            ctx_transpose(h - 1, prev)
                        elif g >= 4:
                            nd_block(h - 1, prev, g - 4)
                if h == 0:
                    # vsum row (eps-scaled) — needs all v blocks
                    ps = psm.tile([128, 512], F32, tag="ps")
                    for tt in range(16):
                        nc.tensor.matmul(ps[0:1, 0:260], o128[:],
                                         vext[:, tt, :],
                                         start=(tt == 0), stop=(tt == 15))
                    nc.vector.tensor_copy(vsre[:], ps[0:1, 0:260])
                # stats first: the 8.5us ekm reduce must beat partB's ph3
                # reduces into the DVE queue, or next cycle's ctx_close
                # rank-1s head-of-line block partA
                stats_chain(h, st)
                if h < nheads - 1:
                    mqc_n = ph3_start(h + 1)
                for i in range(8):
                    eq_block(h, st, i)
                    ctx_block(h, st, i)
                    if h < nheads - 1:
                        ph3_block(h + 1, mqc_n, i)
                if h < nheads - 1:
                    ph3_finish(h + 1, mqc_n)
                if h == 0:
                    tq_half(0)   # mr rows 0 (prologue) + 1 (this cycle)
                elif h == 2:
                    tq_half(1)   # mr rows 2 + 3
                prev = st
            # tail: close/transpose(last) + nd(last) x outproj
            h = nheads - 1
            ctx_close(h, prev)
            ctx_transpose(h, prev)
            if dbg:
                nc.sync.dma_start(dbg_d["d_cs0"], prev["cs"][:])
                nc.sync.dma_start(dbg_d["d_cT0"], prev["cT"][:])
                nc.sync.dma_start(dbg_d["d_ek0"], prev["ek"][:])
                nc.sync.dma_start(dbg_d["d_eq0"], prev["eq"][:])
            if phase < 6:
                for nt in range(4):
                    nd_block(h, prev, nt)
            else:
                nd_block(h, prev, 0)
                nd_block(h, prev, 1)
                op_block(0)
                nd_block(h, prev, 2)
                nd_block(h, prev, 3)
                op_block(1)

            if dbg:
                dr = dp.tile([4, T], F32, tag="dr")
                for i in range(2):
                    nc.sync.dma_start(dr[2 * i:2 * i + 2, :], mrh[i][:])
                nc.sync.dma_start(dbg_d["d_mr"], dr[:])
                dr2 = dp.tile([4, T], F32, tag="dr2")
                for i in range(2):
                    nc.sync.dma_start(dr2[2 * i:2 * i + 2, :], rqh[i][:])
                nc.sync.dma_start(dbg_d["d_rq"], dr2[:])
                dr3 = dp.tile([4, T], BF16, tag="dr3")
                for i in range(2):
                    nc.sync.dma_start(dr3[2 * i:2 * i + 2, :], tqh[i][:])
                nc.sync.dma_start(dbg_d["d_tq"], dr3[:])
                for nm, tile_ in (("d_qt", qt), ("d_kt", kt), ("d_vext", vext),
                                  ("d_dkc", dkc), ("d_vsre", vsre),
                                  ("d_ott", ott)):
                    nc.sync.dma_start(dbg_d[nm], tile_[:])
            sqp.release()
    nc.compile()
    return nc


def _prep_inputs(x, Wq, bq, Wk, bk, Wv, bv, Wo, bo, proj):
    import ml_dtypes
    bf16 = ml_dtypes.bfloat16
    dn = float(D) ** -0.25
    projT_dn = np.ascontiguousarray((dn * proj).T).astype(np.float32)  # [D, M]
    projT2 = np.concatenate([projT_dn, projT_dn], 0).astype(bf16)      # [128, M]
    sel = np.zeros((128, 4), np.float32)
    sel[0:64, 0] = 0.0625
    sel[64:128, 1] = 0.0625
    sel[0:64, 2] = -0.0625
    sel[64:128, 3] = -0.0625
    ident = np.eye(128, dtype=np.float32)
    common = {
        "projT2": projT2,
        "sel": sel.astype(bf16),
        "ident": ident,
    }
    in_maps = []
    for c in range(NCORES):
        b, hg = c // 2, c % 2
        sl = slice(C * hg, C * hg + C)
        bvs = bv[sl]
        bvcol = np.zeros((1, 260), np.float32)
        for h in range(4):
            bvcol[0, 65 * h:65 * h + 64] = bvs[64 * h:64 * h + 64]
        m = dict(common)
        m["xT"] = np.ascontiguousarray(
            x[b].T.reshape(4, 128, T).transpose(1, 0, 2)).astype(bf16)
        m["wq"] = np.ascontiguousarray(
            Wq[:, sl].reshape(4, 128, C).transpose(1, 0, 2)).astype(bf16)
        m["wk"] = np.ascontiguousarray(
            Wk[:, sl].reshape(4, 128, C).transpose(1, 0, 2)).astype(bf16)
        m["wv"] = np.ascontiguousarray(
            Wv[:, sl].reshape(4, 128, C).transpose(1, 0, 2)).astype(bf16)
        m["wo"] = np.ascontiguousarray(
            Wo[sl, :].reshape(2, 128, E).transpose(1, 0, 2)).astype(bf16)
        m["bq"] = np.ascontiguousarray(bq[sl].reshape(C, 1)).astype(np.float32)
        m["bk"] = np.ascontiguousarray(bk[sl].reshape(C, 1)).astype(np.float32)
        m["bvcol"] = bvcol.astype(bf16)
        in_maps.append(m)
    return in_maps


def kernel(x, Wq, bq, Wk, bk, Wv, bv, Wo, bo, proj, _trace=False):
    from concourse.bass_utils import run_bass_kernel_spmd

    x = np.asarray(x, np.float32)
    args = [np.asarray(a, np.float32) for a in (Wq, bq, Wk, bk, Wv, bv, Wo, bo, proj)]
    Wq, bq, Wk, bk, Wv, bv, Wo, bo, proj = args

    if "nc" not in _CACHE:
        _CACHE["nc"] = _build()
    nc = _CACHE["nc"]

    in_maps = _prep_inputs(x, Wq, bq, Wk, bk, Wv, bv, Wo, bo, proj)
    res = run_bass_kernel_spmd(nc, in_maps, list(range(NCORES)), trace=_trace)
    out = np.zeros((4, T, E), np.float32)
    for c in range(NCORES):
        out[c // 2] += np.asarray(res.results[c]["pT"], np.float32).T
    out += bo[None, None, :]
    if _trace:
        return out, res
    return out
